# revision 7
# baseline (speedup 1.0000x reference)
"""
Multi-head masked (causal) attention on 8 Trainium2 NeuronCores.

Sharding: core = 2*b + g  (b = batch 0..3, g = head-group 0..1, 6 heads each).
Each core computes, for its batch b and heads [6g, 6g+6):
    q,k,v projections -> causal attention -> out-projection rows
    [384g, 384g+384) of Wo, output written TRANSPOSED [768, 2048] bf16.
Host gathers: out[b] = (part[2b] + part[2b+1]).T + bo.

Attention is processed per head-PAIR p (3 pairs) and per 512-wide query
tile t (4 tiles).  Scores are computed transposed (S^T[sk, sq]) with the
two heads of a pair occupying PE row-groups 0-1 / 2-3 concurrently
(K=64 each), written into ONE [128, 1024] PSUM tile (half h at columns
[512h, 512h+W)), so a single ACT exp instruction covers both heads.
Causal masking of the 16 diagonal blocks is a DVE multiply of the
exp'd tile by a 0/1 triangle (cheaper than PE mask-matmuls: PE is the
bottleneck engine).  AV uses V with an appended ones-block
([v_h(64) | ones(64)], M=128) so one matmul accumulates both ctx^T and
the softmax denominator; normalization is a per-head reciprocal +
multiply straight out of PSUM (no copies).

PSUM budget (8 banks): score pipeline 3 x [128,1024] f32 (6 banks,
shared with projection/out-proj filler units) + ctx0 + ctx1 (2 banks).

Out-projection: contributions of pairs 0,1 are staged in SBUF (bf16)
as PE filler work; pair-2 contribution is added on top (DVE) and the
single bf16 result is DMA'd out per (t, n-pair) tile.
"""

import numpy as np
import ml_dtypes

import concourse.bass as bass
import concourse.mybir as mybir
import concourse.tile as tile
from concourse import bacc

BF16 = mybir.dt.bfloat16
F32 = mybir.dt.float32

# Problem constants (hardcoded per contract)
B, S, D = 4, 2048, 768
N_HEADS_TOTAL = 12
HD = 64                      # head dim
H = 6                        # local heads per core
NPAIR = H // 2               # head pairs
NC_D = D // 128              # contraction chunks over D (6)
NSK = S // 128               # key blocks (16)
SC = 512                     # query-tile width
NT = S // SC                 # query tiles (4)
SCALE = 1.0 / np.sqrt(HD)


def build_nc():
    nc = bacc.Bacc(None, target_bir_lowering=False)

    xT_d = nc.declare_dram_parameter("xT", [D, S], BF16, isOutput=False)
    wq_d = nc.declare_dram_parameter("wq", [128, NC_D * 384], BF16, isOutput=False)
    wk_d = nc.declare_dram_parameter("wk", [128, NC_D * 384], BF16, isOutput=False)
    wv_d = nc.declare_dram_parameter("wv", [128, NC_D * 384], BF16, isOutput=False)
    wo_d = nc.declare_dram_parameter("wo", [128, 3 * 768], BF16, isOutput=False)
    bqk_d = nc.declare_dram_parameter("bqk", [128, 2 * NPAIR], F32, isOutput=False)
    bv_d = nc.declare_dram_parameter("bv", [128, 384], F32, isOutput=False)
    # [128, 128] lower-triangle keep-mask (tri[sk, sq] = 1 where sq >= sk)
    const_d = nc.declare_dram_parameter("const", [128, 128], BF16, isOutput=False)
    outT_d = nc.declare_dram_parameter("outT", [D, S], BF16, isOutput=True)
    outT_v = outT_d.rearrange("(k p) c -> p k c", p=128)

    with tile.TileContext(nc) as tc:
        with (
            tc.tile_pool(name="const", bufs=1) as constp,
            tc.tile_pool(name="big", bufs=1) as bigp,
            tc.tile_pool(name="epool", bufs=4) as epool,
            tc.tile_pool(name="rpool", bufs=2) as rpool,
            tc.tile_pool(name="opool", bufs=3) as opool,
            tc.tile_pool(name="spool", bufs=3, space="PSUM") as spool,
            tc.tile_pool(name="cpool", bufs=1, space="PSUM") as cpool,
        ):
            xT_sb = bigp.tile([128, NC_D, S], BF16)
            qT_sb = bigp.tile([128, NPAIR, S], BF16)
            kT_sb = bigp.tile([128, NPAIR, S], BF16)
            v_sb = bigp.tile([128, NSK, H * 128], BF16)
            ctxT_sb = bigp.tile([128, NPAIR, S], BF16)
            stage_sb = bigp.tile([128, D // 128, S], BF16)
            wq_sb = constp.tile([128, NC_D, 384], BF16)
            wk_sb = constp.tile([128, NC_D, 384], BF16)
            wv_sb = constp.tile([128, NC_D, 384], BF16)
            wo_sb = constp.tile([128, 3, 768], BF16)
            bqk_sb = constp.tile([128, 2 * NPAIR], F32)
            bv_sb = constp.tile([128, 384], F32)
            tri_sb = constp.tile([128, 128], BF16)

            # ---- input DMAs.  sync: xT (first 512 cols fine-grained, rest
            # bulk).  scalar: weights/consts, most-urgent first.
            for c in range(NC_D):
                nc.sync.dma_start(xT_sb[:, c, 0:512],
                                  xT_d[c * 128:(c + 1) * 128, 0:512])
            nc.scalar.dma_start(wq_sb[:], wq_d.rearrange("p (c n) -> p c n", n=384))
            nc.scalar.dma_start(bqk_sb[:], bqk_d[:])
            nc.scalar.dma_start(tri_sb[:], const_d[:])
            # ones-blocks of v (cols [64,128) per head), set on the idle ACT
            # engine: out = Copy(in*0 + 1) reads its own (uninit) output region
            v_ones = v_sb[:].rearrange("p s (h c) -> p s h c", h=H)[:, :, :, HD:128]
            nc.scalar.activation(v_ones, v_ones,
                                 mybir.ActivationFunctionType.Identity,
                                 bias=1.0, scale=0.0)
            for c in range(NC_D):
                nc.sync.dma_start(xT_sb[:, c, 512:1024],
                                  xT_d[c * 128:(c + 1) * 128, 512:1024])
            for c in range(NC_D):
                nc.sync.dma_start(xT_sb[:, c, 1024:S],
                                  xT_d[c * 128:(c + 1) * 128, 1024:S])
            nc.scalar.dma_start(wk_sb[:], wk_d.rearrange("p (c n) -> p c n", n=384))
            nc.scalar.dma_start(wv_sb[:], wv_d.rearrange("p (c n) -> p c n", n=384))
            nc.scalar.dma_start(bv_sb[:], bv_d[:])
            nc.scalar.dma_start(wo_sb[:], wo_d.rearrange("p (c n) -> p c n", n=768))

            # ---- projection / out-projection units (PE filler work) ----
            def qk_sub(p, which, t):
                """q or k projection for pair p, 512 query cols."""
                w_sb, dst = ((wq_sb, qT_sb), (wk_sb, kT_sb))[which]
                ps = spool.tile([128, 1024], F32, tag="work", name=f"qks{p}{which}{t}")
                for c in range(NC_D):
                    nc.tensor.matmul(
                        ps[:, 0:512],
                        w_sb[:, c, p * 128:(p + 1) * 128],
                        xT_sb[:, c, t * 512:(t + 1) * 512],
                        start=(c == 0), stop=(c == NC_D - 1),
                    )
                nc.vector.tensor_add(
                    out=dst[:, p, t * 512:(t + 1) * 512],
                    in0=ps[:, 0:512],
                    in1=bqk_sb[:, 3 * which + p:3 * which + p + 1]
                        .broadcast_to((128, 512)),
                )

            def qk_unit(p, which, tp):
                """q or k projection for pair p, 1024 query cols (2 chunks)."""
                w_sb, dst = ((wq_sb, qT_sb), (wk_sb, kT_sb))[which]
                ps = spool.tile([128, 1024], F32, tag="work", name=f"qk{p}{which}{tp}")
                base = 1024 * tp
                for i in range(2):
                    for c in range(NC_D):
                        nc.tensor.matmul(
                            ps[:, i * 512:(i + 1) * 512],
                            w_sb[:, c, p * 128:(p + 1) * 128],
                            xT_sb[:, c, base + i * 512:base + i * 512 + 512],
                            start=(c == 0), stop=(c == NC_D - 1),
                        )
                nc.vector.tensor_add(
                    out=dst[:, p, base:base + 1024],
                    in0=ps[:, 0:1024],
                    in1=bqk_sb[:, 3 * which + p:3 * which + p + 1]
                        .broadcast_to((128, 1024)),
                )

            def v_unit(j):
                """v projection for token blocks 2j, 2j+1."""
                ps = spool.tile([128, 1024], F32, tag="work", name=f"v{j}")
                for i in range(2):
                    s = 2 * j + i
                    for c in range(NC_D):
                        nc.tensor.matmul(
                            ps[:, i * 512:i * 512 + 384],
                            xT_sb[:, c, s * 128:(s + 1) * 128],
                            wv_sb[:, c, :],
                            start=(c == 0), stop=(c == NC_D - 1),
                        )
                for i in range(2):
                    nc.vector.tensor_add(
                        out=v_sb[:, 2 * j + i, :]
                            .rearrange("p (h c) -> p h c", h=H)[:, :, 0:HD],
                        in0=ps[:, i * 512:i * 512 + 384]
                            .rearrange("p (h c) -> p h c", h=H),
                        in1=bv_sb.rearrange("p (h c) -> p h c", h=H),
                    )

            def o01_unit(t, k):
                """stage out-proj contribution of pairs 0,1 for n = 2k, 2k+1."""
                ps = spool.tile([128, 1024], F32, tag="work", name=f"o01_{t}{k}")
                for i in range(2):
                    n = 2 * k + i
                    for c in range(2):
                        nc.tensor.matmul(
                            ps[:, i * 512:(i + 1) * 512],
                            wo_sb[:, c, n * 128:(n + 1) * 128],
                            ctxT_sb[:, c, t * 512:(t + 1) * 512],
                            start=(c == 0), stop=(c == 1),
                        )
                nc.vector.tensor_copy(
                    stage_sb[:, 2 * k:2 * k + 2, t * 512:(t + 1) * 512],
                    ps.rearrange("p (i c) -> p i c", i=2),
                )

            def o2_unit(t, k):
                """add pair-2 contribution onto the stage and DMA out."""
                ps = spool.tile([128, 1024], F32, tag="work", name=f"o2_{t}{k}")
                for i in range(2):
                    n = 2 * k + i
                    nc.tensor.matmul(
                        ps[:, i * 512:(i + 1) * 512],
                        wo_sb[:, 2, n * 128:(n + 1) * 128],
                        ctxT_sb[:, 2, t * 512:(t + 1) * 512],
                        start=True, stop=True,
                    )
                ot = opool.tile([128, 2, 512], BF16, tag="ot", name=f"ot{t}{k}")
                nc.vector.tensor_add(
                    out=ot[:],
                    in0=ps.rearrange("p (i c) -> p i c", i=2),
                    in1=stage_sb[:, 2 * k:2 * k + 2, t * 512:(t + 1) * 512],
                )
                eng = nc.sync if k % 2 == 0 else nc.scalar
                eng.dma_start(
                    outT_v[:, 2 * k:2 * k + 2, t * 512:(t + 1) * 512],
                    ot[:],
                )

            # ---- attention for pair p, query tile t ----
            def attention(p, t, fillers):
                q0 = SC * t
                nik = 4 * (t + 1)
                ctxs = [cpool.tile([128, 512], F32, tag=f"ctx{hf}", name=f"C{hf}_{p}{t}")
                        for hf in range(2)]

                def emit_scores(ik):
                    sq0 = max(q0, 128 * ik)
                    W = q0 + SC - sq0
                    diag = 128 * ik >= q0
                    s_ps = spool.tile([128, 1024], F32, tag="work",
                                      name=f"s{p}{t}{ik}")
                    for half in range(2):
                        hp = slice(64 * half, 64 * half + 64)
                        nc.tensor.matmul(
                            s_ps[:, 512 * half:512 * half + W],
                            kT_sb[hp, p, ik * 128:(ik + 1) * 128],
                            qT_sb[hp, p, sq0:sq0 + W],
                            start=True, stop=True,
                            skip_group_check=True,
                        )
                    e = epool.tile([128, 1024], BF16, tag="e", name=f"e{p}{t}{ik}")
                    nc.scalar.activation(
                        e.rearrange("p (h c) -> p h c", h=2)[:, :, 0:W],
                        s_ps.rearrange("p (h c) -> p h c", h=2)[:, :, 0:W],
                        mybir.ActivationFunctionType.Exp, scale=float(SCALE),
                    )
                    if diag:
                        # diagonal block: zero the causally-masked entries by
                        # a 0/1-triangle multiply on DVE (PE is the bottleneck)
                        for half in range(2):
                            ev = e[:, 512 * half:512 * half + 128]
                            nc.vector.tensor_mul(ev, ev, tri_sb[:])
                    return e

                e_next = emit_scores(0)
                for ik in range(nik):
                    e = e_next
                    if ik + 1 < nik:
                        e_next = emit_scores(ik + 1)
                    if fillers:
                        npop = -(-len(fillers) // (nik - ik))
                        for _ in range(npop):
                            fillers.pop(0)()
                    sq0 = max(q0, 128 * ik)
                    W = q0 + SC - sq0
                    off = sq0 - q0
                    for half in range(2):
                        h = 2 * p + half
                        nc.tensor.matmul(
                            ctxs[half][:, off:off + W],
                            v_sb[:, ik, 128 * h:128 * (h + 1)],
                            e[:, 512 * half:512 * half + W],
                            start=(ik == 0), stop=(ik == nik - 1),
                            skip_group_check=True,
                        )
                # normalize: ctxT = un/denom per half (SBUF bounce: custom-DVE
                # recip and partition-shifted muls are only proven on SBUF)
                for half in range(2):
                    hp = slice(64 * half, 64 * half + 64)
                    un = rpool.tile([64, 512], F32, tag="un", name=f"u{p}{t}{half}")
                    den = rpool.tile([64, 512], F32, tag="den", name=f"d{p}{t}{half}")
                    nc.vector.tensor_copy(un[:], ctxs[half][0:64, :])
                    nc.vector.tensor_copy(den[:], ctxs[half][64:128, :])
                    rcp = rpool.tile([64, 512], F32, tag="rcp", name=f"r{p}{t}{half}")
                    nc.vector.reciprocal_approx_fast(rcp[:], den[:])
                    nc.vector.tensor_mul(ctxT_sb[hp, p, q0:q0 + SC], un[:], rcp[:])

            # ---- emission schedule ----
            import functools as ft
            P = ft.partial

            # prologue: minimum needed by attention(0,0) / early (0,1)
            qk_sub(0, 0, 0)
            qk_sub(0, 1, 0)
            v_unit(0)
            v_unit(1)

            FILL = {
                (0, 0): [P(qk_sub, 0, 0, 1), P(qk_sub, 0, 1, 1),
                         P(v_unit, 2), P(v_unit, 3)],
                (0, 1): [P(qk_unit, 0, 0, 1), P(qk_unit, 0, 1, 1), P(v_unit, 4)],
                (0, 2): [P(v_unit, 5), P(v_unit, 6), P(v_unit, 7),
                         P(qk_unit, 1, 0, 0)],
                (0, 3): [P(qk_unit, 1, 1, 0), P(qk_unit, 1, 0, 1),
                         P(qk_unit, 1, 1, 1)],
                (1, 0): [P(qk_unit, 2, 0, 0)],
                (1, 1): [P(qk_unit, 2, 1, 0), P(qk_unit, 2, 0, 1)],
                (1, 2): [P(qk_unit, 2, 1, 1), P(o01_unit, 0, 0),
                         P(o01_unit, 0, 1), P(o01_unit, 0, 2)],
                (1, 3): [P(o01_unit, 1, 0), P(o01_unit, 1, 1),
                         P(o01_unit, 1, 2), P(o01_unit, 2, 0)],
                (2, 0): [P(o01_unit, 2, 1), P(o01_unit, 2, 2)],
                (2, 1): [P(o2_unit, 0, 0), P(o2_unit, 0, 1), P(o2_unit, 0, 2),
                         P(o01_unit, 3, 0), P(o01_unit, 3, 1)],
                (2, 2): [P(o01_unit, 3, 2), P(o2_unit, 1, 0),
                         P(o2_unit, 1, 1), P(o2_unit, 1, 2)],
                (2, 3): [P(o2_unit, 2, 0), P(o2_unit, 2, 1), P(o2_unit, 2, 2)],
            }
            for p in range(NPAIR):
                for t in range(NT):
                    attention(p, t, FILL[(p, t)])
            # tail: pair-2 contribution for the last query tile
            o2_unit(3, 0)
            o2_unit(3, 1)
            o2_unit(3, 2)
    nc.finalize()
    return nc


_NC_CACHE = None


def _get_nc():
    global _NC_CACHE
    if _NC_CACHE is None:
        _NC_CACHE = build_nc()
    return _NC_CACHE


def make_in_maps(x, Wq, Wk, Wv, bq, bk, bv, Wo, bo):
    bf16 = ml_dtypes.bfloat16
    # tri[sk, sq] = 1 where sq >= sk (keep), 0 where causally masked
    const = np.ascontiguousarray(
        np.where(np.arange(128)[None, :] >= np.arange(128)[:, None],
                 np.float32(1.0), np.float32(0.0))).astype(bf16)

    def prep_w(W, hs):
        # [H, 768, 64] -> [768, H*64] -> [128, NC_D, 384] -> [128, NC_D*384]
        w = np.asarray(W[hs]).transpose(1, 0, 2).reshape(D, H * HD)
        w = w.reshape(NC_D, 128, H * HD).transpose(1, 0, 2).reshape(128, -1)
        return np.ascontiguousarray(w).astype(bf16)

    in_maps = []
    for core in range(8):
        b, g = core // 2, core % 2
        hs = slice(6 * g, 6 * g + 6)
        xT = np.ascontiguousarray(np.asarray(x[b]).T).astype(bf16)
        bqk = np.zeros((128, 2 * NPAIR), np.float32)
        for p in range(NPAIR):
            bqk[0:64, p] = bq[6 * g + 2 * p]
            bqk[64:128, p] = bq[6 * g + 2 * p + 1]
            bqk[0:64, NPAIR + p] = bk[6 * g + 2 * p]
            bqk[64:128, NPAIR + p] = bk[6 * g + 2 * p + 1]
        bvr = np.ascontiguousarray(
            np.broadcast_to(np.asarray(bv[hs]).reshape(1, H * HD), (128, H * HD))
        ).astype(np.float32)
        wo = np.asarray(Wo[384 * g:384 * (g + 1), :])
        wo = wo.reshape(3, 128, D).transpose(1, 0, 2).reshape(128, -1)
        wo = np.ascontiguousarray(wo).astype(bf16)
        in_maps.append({
            "xT": xT,
            "wq": prep_w(Wq, hs), "wk": prep_w(Wk, hs), "wv": prep_w(Wv, hs),
            "wo": wo, "bqk": bqk, "bv": bvr, "const": const,
        })
    return in_maps


def gather_out(results, bo):
    out = np.empty((B, S, D), np.float32)
    bo32 = np.asarray(bo, np.float32)
    for b in range(B):
        pT = (results[2 * b]["outT"].astype(np.float32)
              + results[2 * b + 1]["outT"].astype(np.float32))
        out[b] = pT.T + bo32[None, :]
    return out


def kernel(x, Wq, Wk, Wv, bq, bk, bv, Wo, bo):
    from concourse.bass_utils import run_bass_kernel_spmd

    nc = _get_nc()
    in_maps = make_in_maps(x, Wq, Wk, Wv, bq, bk, bv, Wo, bo)
    res = run_bass_kernel_spmd(nc, in_maps, list(range(8)))
    return gather_out(res.results, bo)


# revision 8
# speedup vs baseline: 1.0087x; 1.0087x over previous
"""
Multi-head masked (causal) attention on 8 Trainium2 NeuronCores.

Sharding: core = 2*b + g  (b = batch 0..3, g = head-group 0..1, 6 heads each).
Each core computes, for its batch b and heads [6g, 6g+6):
    q,k,v projections -> causal attention -> out-projection rows
    [384g, 384g+384) of Wo, output written TRANSPOSED [768, 2048] bf16.
Host gathers: out[b] = (part[2b] + part[2b+1]).T + bo.

Attention is processed per head-PAIR p (3 pairs) and per 512-wide query
tile t (4 tiles).  Scores are computed transposed (S^T[sk, sq]) with the
two heads of a pair occupying PE row-groups 0-1 / 2-3 concurrently
(K=64 each), written into ONE [128, 1024] PSUM tile (half h at columns
[512h, 512h+W)), so a single ACT exp instruction covers both heads.
Causal masking of the 16 diagonal blocks is a DVE multiply of the
exp'd tile by a 0/1 triangle (cheaper than PE mask-matmuls: PE is the
bottleneck engine).  AV uses V with an appended ones-block
([v_h(64) | ones(64)], M=128) so one matmul accumulates both ctx^T and
the softmax denominator; normalization is a per-head reciprocal +
multiply straight out of PSUM (no copies).

PSUM budget (8 banks): score pipeline 3 x [128,1024] f32 (6 banks,
shared with projection/out-proj filler units) + ctx0 + ctx1 (2 banks).

Out-projection: contributions of pairs 0,1 are staged in SBUF (bf16)
as PE filler work; pair-2 contribution is added on top (DVE) and the
single bf16 result is DMA'd out per (t, n-pair) tile.
"""

import numpy as np
import ml_dtypes

import concourse.bass as bass
import concourse.mybir as mybir
import concourse.tile as tile
from concourse import bacc

BF16 = mybir.dt.bfloat16
F32 = mybir.dt.float32

# Problem constants (hardcoded per contract)
B, S, D = 4, 2048, 768
N_HEADS_TOTAL = 12
HD = 64                      # head dim
H = 6                        # local heads per core
NPAIR = H // 2               # head pairs
NC_D = D // 128              # contraction chunks over D (6)
NSK = S // 128               # key blocks (16)
SC = 512                     # query-tile width
NT = S // SC                 # query tiles (4)
SCALE = 1.0 / np.sqrt(HD)


def build_nc():
    nc = bacc.Bacc(None, target_bir_lowering=False)

    xT_d = nc.declare_dram_parameter("xT", [D, S], BF16, isOutput=False)
    wq_d = nc.declare_dram_parameter("wq", [128, NC_D * 384], BF16, isOutput=False)
    wk_d = nc.declare_dram_parameter("wk", [128, NC_D * 384], BF16, isOutput=False)
    wv_d = nc.declare_dram_parameter("wv", [128, NC_D * 384], BF16, isOutput=False)
    wo_d = nc.declare_dram_parameter("wo", [128, 3 * 768], BF16, isOutput=False)
    bqk_d = nc.declare_dram_parameter("bqk", [128, 2 * NPAIR], F32, isOutput=False)
    bv_d = nc.declare_dram_parameter("bv", [128, 384], F32, isOutput=False)
    # [128, 128] lower-triangle keep-mask (tri[sk, sq] = 1 where sq >= sk)
    const_d = nc.declare_dram_parameter("const", [128, 128], BF16, isOutput=False)
    outT_d = nc.declare_dram_parameter("outT", [D, S], BF16, isOutput=True)
    outT_v = outT_d.rearrange("(k p) c -> p k c", p=128)

    with tile.TileContext(nc) as tc:
        with (
            tc.tile_pool(name="const", bufs=1) as constp,
            tc.tile_pool(name="big", bufs=1) as bigp,
            tc.tile_pool(name="epool", bufs=4) as epool,
            tc.tile_pool(name="rpool", bufs=2) as rpool,
            tc.tile_pool(name="opool", bufs=3) as opool,
            tc.tile_pool(name="spool", bufs=3, space="PSUM") as spool,
            tc.tile_pool(name="cpool", bufs=1, space="PSUM") as cpool,
        ):
            xT_sb = bigp.tile([128, NC_D, S], BF16)
            qT_sb = bigp.tile([128, NPAIR, S], BF16)
            kT_sb = bigp.tile([128, NPAIR, S], BF16)
            v_sb = bigp.tile([128, NSK, H * 128], BF16)
            ctxT_sb = bigp.tile([128, NPAIR, S], BF16)
            stage_sb = bigp.tile([128, D // 128, S], BF16)
            wq_sb = constp.tile([128, NC_D, 384], BF16)
            wk_sb = constp.tile([128, NC_D, 384], BF16)
            wv_sb = constp.tile([128, NC_D, 384], BF16)
            wo_sb = constp.tile([128, 3, 768], BF16)
            bqk_sb = constp.tile([128, 2 * NPAIR], F32)
            bv_sb = constp.tile([128, 384], F32)
            tri_sb = constp.tile([128, 128], BF16)

            # ---- input DMAs.  sync: xT (first 512 cols fine-grained, rest
            # bulk).  scalar: weights/consts, most-urgent first.
            for c in range(NC_D):
                nc.sync.dma_start(xT_sb[:, c, 0:512],
                                  xT_d[c * 128:(c + 1) * 128, 0:512])
            nc.scalar.dma_start(wq_sb[:], wq_d.rearrange("p (c n) -> p c n", n=384))
            nc.scalar.dma_start(bqk_sb[:], bqk_d[:])
            nc.scalar.dma_start(tri_sb[:], const_d[:])
            # ones-blocks of v (cols [64,128) per head), set once on the
            # otherwise-idle GPSIMD engine (keeps ACT free for the exp stream)
            v_ones = v_sb[:].rearrange("p s (h c) -> p s h c", h=H)[:, :, :, HD:128]
            nc.gpsimd.memset(v_ones, 1.0)
            for c in range(NC_D):
                nc.sync.dma_start(xT_sb[:, c, 512:1024],
                                  xT_d[c * 128:(c + 1) * 128, 512:1024])
            for c in range(NC_D):
                nc.sync.dma_start(xT_sb[:, c, 1024:S],
                                  xT_d[c * 128:(c + 1) * 128, 1024:S])
            nc.scalar.dma_start(wk_sb[:], wk_d.rearrange("p (c n) -> p c n", n=384))
            nc.scalar.dma_start(wv_sb[:], wv_d.rearrange("p (c n) -> p c n", n=384))
            nc.scalar.dma_start(bv_sb[:], bv_d[:])
            nc.scalar.dma_start(wo_sb[:], wo_d.rearrange("p (c n) -> p c n", n=768))

            # ---- projection / out-projection units (PE filler work) ----
            def qk_sub(p, which, t):
                """q or k projection for pair p, 512 query cols."""
                w_sb, dst = ((wq_sb, qT_sb), (wk_sb, kT_sb))[which]
                ps = spool.tile([128, 1024], F32, tag="work", name=f"qks{p}{which}{t}")
                for c in range(NC_D):
                    nc.tensor.matmul(
                        ps[:, 0:512],
                        w_sb[:, c, p * 128:(p + 1) * 128],
                        xT_sb[:, c, t * 512:(t + 1) * 512],
                        start=(c == 0), stop=(c == NC_D - 1),
                    )
                nc.vector.tensor_add(
                    out=dst[:, p, t * 512:(t + 1) * 512],
                    in0=ps[:, 0:512],
                    in1=bqk_sb[:, 3 * which + p:3 * which + p + 1]
                        .broadcast_to((128, 512)),
                )

            def qk_unit(p, which, tp):
                """q or k projection for pair p, 1024 query cols (2 chunks)."""
                w_sb, dst = ((wq_sb, qT_sb), (wk_sb, kT_sb))[which]
                ps = spool.tile([128, 1024], F32, tag="work", name=f"qk{p}{which}{tp}")
                base = 1024 * tp
                for i in range(2):
                    for c in range(NC_D):
                        nc.tensor.matmul(
                            ps[:, i * 512:(i + 1) * 512],
                            w_sb[:, c, p * 128:(p + 1) * 128],
                            xT_sb[:, c, base + i * 512:base + i * 512 + 512],
                            start=(c == 0), stop=(c == NC_D - 1),
                        )
                nc.vector.tensor_add(
                    out=dst[:, p, base:base + 1024],
                    in0=ps[:, 0:1024],
                    in1=bqk_sb[:, 3 * which + p:3 * which + p + 1]
                        .broadcast_to((128, 1024)),
                )

            def v_unit(j):
                """v projection for token blocks 2j, 2j+1."""
                ps = spool.tile([128, 1024], F32, tag="work", name=f"v{j}")
                for i in range(2):
                    s = 2 * j + i
                    for c in range(NC_D):
                        nc.tensor.matmul(
                            ps[:, i * 512:i * 512 + 384],
                            xT_sb[:, c, s * 128:(s + 1) * 128],
                            wv_sb[:, c, :],
                            start=(c == 0), stop=(c == NC_D - 1),
                        )
                for i in range(2):
                    nc.vector.tensor_add(
                        out=v_sb[:, 2 * j + i, :]
                            .rearrange("p (h c) -> p h c", h=H)[:, :, 0:HD],
                        in0=ps[:, i * 512:i * 512 + 384]
                            .rearrange("p (h c) -> p h c", h=H),
                        in1=bv_sb.rearrange("p (h c) -> p h c", h=H),
                    )

            def o01_unit(t, k):
                """stage out-proj contribution of pairs 0,1 for n = 2k, 2k+1."""
                ps = spool.tile([128, 1024], F32, tag="work", name=f"o01_{t}{k}")
                for i in range(2):
                    n = 2 * k + i
                    for c in range(2):
                        nc.tensor.matmul(
                            ps[:, i * 512:(i + 1) * 512],
                            wo_sb[:, c, n * 128:(n + 1) * 128],
                            ctxT_sb[:, c, t * 512:(t + 1) * 512],
                            start=(c == 0), stop=(c == 1),
                        )
                nc.vector.tensor_copy(
                    stage_sb[:, 2 * k:2 * k + 2, t * 512:(t + 1) * 512],
                    ps.rearrange("p (i c) -> p i c", i=2),
                )

            def o2_unit(t, k):
                """add pair-2 contribution onto the stage and DMA out."""
                ps = spool.tile([128, 1024], F32, tag="work", name=f"o2_{t}{k}")
                for i in range(2):
                    n = 2 * k + i
                    nc.tensor.matmul(
                        ps[:, i * 512:(i + 1) * 512],
                        wo_sb[:, 2, n * 128:(n + 1) * 128],
                        ctxT_sb[:, 2, t * 512:(t + 1) * 512],
                        start=True, stop=True,
                    )
                ot = opool.tile([128, 2, 512], BF16, tag="ot", name=f"ot{t}{k}")
                nc.vector.tensor_add(
                    out=ot[:],
                    in0=ps.rearrange("p (i c) -> p i c", i=2),
                    in1=stage_sb[:, 2 * k:2 * k + 2, t * 512:(t + 1) * 512],
                )
                eng = nc.sync if k % 2 == 0 else nc.scalar
                eng.dma_start(
                    outT_v[:, 2 * k:2 * k + 2, t * 512:(t + 1) * 512],
                    ot[:],
                )

            # ---- attention for pair p, query tile t ----
            def attention(p, t, fillers):
                q0 = SC * t
                nik = 4 * (t + 1)
                ctxs = [cpool.tile([128, 512], F32, tag=f"ctx{hf}", name=f"C{hf}_{p}{t}")
                        for hf in range(2)]

                def emit_scores(ik):
                    sq0 = max(q0, 128 * ik)
                    W = q0 + SC - sq0
                    diag = 128 * ik >= q0
                    s_ps = spool.tile([128, 1024], F32, tag="work",
                                      name=f"s{p}{t}{ik}")
                    for half in range(2):
                        hp = slice(64 * half, 64 * half + 64)
                        nc.tensor.matmul(
                            s_ps[:, 512 * half:512 * half + W],
                            kT_sb[hp, p, ik * 128:(ik + 1) * 128],
                            qT_sb[hp, p, sq0:sq0 + W],
                            start=True, stop=True,
                            skip_group_check=True,
                        )
                    e = epool.tile([128, 1024], BF16, tag="e", name=f"e{p}{t}{ik}")
                    nc.scalar.activation(
                        e.rearrange("p (h c) -> p h c", h=2)[:, :, 0:W],
                        s_ps.rearrange("p (h c) -> p h c", h=2)[:, :, 0:W],
                        mybir.ActivationFunctionType.Exp, scale=float(SCALE),
                    )
                    if diag:
                        # diagonal block: zero the causally-masked entries by
                        # a 0/1-triangle multiply on the idle GPSIMD engine
                        # (PE is the bottleneck, DVE queue would stall the
                        # exp->AV chain)
                        for half in range(2):
                            ev = e[:, 512 * half:512 * half + 128]
                            nc.gpsimd.tensor_mul(ev, ev, tri_sb[:])
                    return e

                e_next = emit_scores(0)
                for ik in range(nik):
                    e = e_next
                    if ik + 1 < nik:
                        e_next = emit_scores(ik + 1)
                    if fillers:
                        npop = -(-len(fillers) // (nik - ik))
                        for _ in range(npop):
                            fillers.pop(0)()
                    sq0 = max(q0, 128 * ik)
                    W = q0 + SC - sq0
                    off = sq0 - q0
                    for half in range(2):
                        h = 2 * p + half
                        nc.tensor.matmul(
                            ctxs[half][:, off:off + W],
                            v_sb[:, ik, 128 * h:128 * (h + 1)],
                            e[:, 512 * half:512 * half + W],
                            start=(ik == 0), stop=(ik == nik - 1),
                            skip_group_check=True,
                        )
                # normalize: ctxT = un/denom per half (SBUF bounce: custom-DVE
                # recip and partition-shifted muls are only proven on SBUF)
                for half in range(2):
                    hp = slice(64 * half, 64 * half + 64)
                    un = rpool.tile([64, 512], F32, tag="un", name=f"u{p}{t}{half}")
                    den = rpool.tile([64, 512], F32, tag="den", name=f"d{p}{t}{half}")
                    nc.vector.tensor_copy(un[:], ctxs[half][0:64, :])
                    nc.vector.tensor_copy(den[:], ctxs[half][64:128, :])
                    rcp = rpool.tile([64, 512], F32, tag="rcp", name=f"r{p}{t}{half}")
                    nc.vector.reciprocal_approx_fast(rcp[:], den[:])
                    nc.vector.tensor_mul(ctxT_sb[hp, p, q0:q0 + SC], un[:], rcp[:])

            # ---- emission schedule ----
            import functools as ft
            P = ft.partial

            # prologue: minimum needed by attention(0,0) / early (0,1)
            qk_sub(0, 0, 0)
            qk_sub(0, 1, 0)
            v_unit(0)
            v_unit(1)

            FILL = {
                (0, 0): [P(qk_sub, 0, 0, 1), P(qk_sub, 0, 1, 1),
                         P(v_unit, 2), P(v_unit, 3)],
                (0, 1): [P(qk_unit, 0, 0, 1), P(qk_unit, 0, 1, 1), P(v_unit, 4)],
                (0, 2): [P(v_unit, 5), P(v_unit, 6), P(v_unit, 7),
                         P(qk_unit, 1, 0, 0)],
                (0, 3): [P(qk_unit, 1, 1, 0), P(qk_unit, 1, 0, 1),
                         P(qk_unit, 1, 1, 1)],
                (1, 0): [P(qk_unit, 2, 0, 0)],
                (1, 1): [P(qk_unit, 2, 1, 0), P(qk_unit, 2, 0, 1)],
                (1, 2): [P(qk_unit, 2, 1, 1), P(o01_unit, 0, 0),
                         P(o01_unit, 0, 1), P(o01_unit, 0, 2)],
                (1, 3): [P(o01_unit, 1, 0), P(o01_unit, 1, 1),
                         P(o01_unit, 1, 2), P(o01_unit, 2, 0)],
                (2, 0): [P(o01_unit, 2, 1), P(o01_unit, 2, 2)],
                (2, 1): [P(o2_unit, 0, 0), P(o2_unit, 0, 1), P(o2_unit, 0, 2),
                         P(o01_unit, 3, 0), P(o01_unit, 3, 1)],
                (2, 2): [P(o01_unit, 3, 2), P(o2_unit, 1, 0),
                         P(o2_unit, 1, 1), P(o2_unit, 1, 2)],
                (2, 3): [P(o2_unit, 2, 0), P(o2_unit, 2, 1), P(o2_unit, 2, 2)],
            }
            for p in range(NPAIR):
                for t in range(NT):
                    attention(p, t, FILL[(p, t)])
            # tail: pair-2 contribution for the last query tile
            o2_unit(3, 0)
            o2_unit(3, 1)
            o2_unit(3, 2)
    nc.finalize()
    return nc


_NC_CACHE = None


def _get_nc():
    global _NC_CACHE
    if _NC_CACHE is None:
        _NC_CACHE = build_nc()
    return _NC_CACHE


def make_in_maps(x, Wq, Wk, Wv, bq, bk, bv, Wo, bo):
    bf16 = ml_dtypes.bfloat16
    # tri[sk, sq] = 1 where sq >= sk (keep), 0 where causally masked
    const = np.ascontiguousarray(
        np.where(np.arange(128)[None, :] >= np.arange(128)[:, None],
                 np.float32(1.0), np.float32(0.0))).astype(bf16)

    def prep_w(W, hs):
        # [H, 768, 64] -> [768, H*64] -> [128, NC_D, 384] -> [128, NC_D*384]
        w = np.asarray(W[hs]).transpose(1, 0, 2).reshape(D, H * HD)
        w = w.reshape(NC_D, 128, H * HD).transpose(1, 0, 2).reshape(128, -1)
        return np.ascontiguousarray(w).astype(bf16)

    in_maps = []
    for core in range(8):
        b, g = core // 2, core % 2
        hs = slice(6 * g, 6 * g + 6)
        xT = np.ascontiguousarray(np.asarray(x[b]).T).astype(bf16)
        bqk = np.zeros((128, 2 * NPAIR), np.float32)
        for p in range(NPAIR):
            bqk[0:64, p] = bq[6 * g + 2 * p]
            bqk[64:128, p] = bq[6 * g + 2 * p + 1]
            bqk[0:64, NPAIR + p] = bk[6 * g + 2 * p]
            bqk[64:128, NPAIR + p] = bk[6 * g + 2 * p + 1]
        bvr = np.ascontiguousarray(
            np.broadcast_to(np.asarray(bv[hs]).reshape(1, H * HD), (128, H * HD))
        ).astype(np.float32)
        wo = np.asarray(Wo[384 * g:384 * (g + 1), :])
        wo = wo.reshape(3, 128, D).transpose(1, 0, 2).reshape(128, -1)
        wo = np.ascontiguousarray(wo).astype(bf16)
        in_maps.append({
            "xT": xT,
            "wq": prep_w(Wq, hs), "wk": prep_w(Wk, hs), "wv": prep_w(Wv, hs),
            "wo": wo, "bqk": bqk, "bv": bvr, "const": const,
        })
    return in_maps


def gather_out(results, bo):
    out = np.empty((B, S, D), np.float32)
    bo32 = np.asarray(bo, np.float32)
    for b in range(B):
        pT = (results[2 * b]["outT"].astype(np.float32)
              + results[2 * b + 1]["outT"].astype(np.float32))
        out[b] = pT.T + bo32[None, :]
    return out


def kernel(x, Wq, Wk, Wv, bq, bk, bv, Wo, bo):
    from concourse.bass_utils import run_bass_kernel_spmd

    nc = _get_nc()
    in_maps = make_in_maps(x, Wq, Wk, Wv, bq, bk, bv, Wo, bo)
    res = run_bass_kernel_spmd(nc, in_maps, list(range(8)))
    return gather_out(res.results, bo)
